# revision 7
# baseline (speedup 1.0000x reference)
"""Trainium2 Bass kernel: BasicMultiheadAttention (B=2, S=2048, D=1024, H=16).

Sharding: tensor-parallel over heads. Core c owns heads (2c, 2c+1) for both
batches; computes QKV^T, attention (scores^T layout, softmax via exp +
ones-augmented-V denominator), normalized ctx^T; AllGather of ctx^T across the
8 cores; then column-sharded output projection (+bias) per core.

Host side: transpose/cast inputs to fp16, shard weights per core, and
re-assemble the output transpose at the end.
"""

import numpy as np

B, S, D, H = 2, 2048, 1024, 16
DH = D // H  # 64
NCORES = 8
HPC = H // NCORES  # heads per core = 2
SQ = B * S  # 4096 tokens
NKT = D // 128  # 8 contraction k-tiles over D
KT_S = S // 128  # 16 key tiles per batch
QC_S = S // 512  # 4 query chunks of 512 per batch

_CACHE = {}


def _ensure_axon_hooks():
    """This image's antenv lacks axon_hooks; bass_utils imports it when
    trace=True under axon. Register an equivalent stub backed by the boot
    helper so NTFF profiling works (or degrades gracefully)."""
    import sys
    import types
    try:
        import antenv.axon_hooks  # noqa: F401
        return
    except ImportError:
        pass
    try:
        import antenv
        hook = [None]
        try:
            from trn_agent_boot.trn_boot import _ntff_profile_via_ctypes
            hook[0] = _ntff_profile_via_ctypes("/opt/axon/libaxon_pjrt.so")
        except Exception:
            hook[0] = None
        mod = types.ModuleType("antenv.axon_hooks")
        mod.get_axon_ntff_profile_hook = lambda: hook[0]
        mod.set_axon_ntff_profile_hook = lambda h: hook.__setitem__(0, h)
        sys.modules["antenv.axon_hooks"] = mod
        antenv.axon_hooks = mod
    except Exception:
        pass


_ensure_axon_hooks()


def _build_kernel():
    import concourse.bass as bass  # noqa: F401
    import concourse.mybir as mybir
    import concourse.tile as tile
    from concourse import bacc
    from concourse.masks import make_identity

    f16 = mybir.dt.float16
    f32 = mybir.dt.float32
    AF = mybir.ActivationFunctionType

    nc = bacc.Bacc(None, num_devices=NCORES)

    # ---- I/O ----
    xT = nc.dram_tensor("xT", [D, SQ], f16, kind="ExternalInput")
    wq = nc.dram_tensor("wq", [128, D], f16, kind="ExternalInput")
    wk = nc.dram_tensor("wk", [128, D], f16, kind="ExternalInput")
    wv = nc.dram_tensor("wv", [128, D], f16, kind="ExternalInput")
    wo = nc.dram_tensor("wo", [128, D], f16, kind="ExternalInput")
    bq = nc.dram_tensor("bq", [128, 1], f32, kind="ExternalInput")
    bk = nc.dram_tensor("bk", [128, 1], f32, kind="ExternalInput")
    bvb = nc.dram_tensor("bvb", [128, 128], f32, kind="ExternalInput")
    bo = nc.dram_tensor("bo", [128, 1], f32, kind="ExternalInput")
    yT = nc.dram_tensor("yT", [128, SQ], f32, kind="ExternalOutput")

    with tile.TileContext(nc) as tc:
        with (
            tc.tile_pool(name="const", bufs=1) as const,
            tc.tile_pool(name="psA", bufs=2, space="PSUM") as psA,   # qkv/outproj/transpose
            tc.tile_pool(name="psSc", bufs=2, space="PSUM") as psSc,  # scores
            tc.tile_pool(name="psCtx", bufs=2, space="PSUM") as psCtx,  # ctx accum
            tc.tile_pool(name="pP", bufs=4) as pP,
            tc.tile_pool(name="pSmall", bufs=4) as pSmall,
            tc.tile_pool(name="pCtxn", bufs=4) as pCtxn,
            tc.tile_pool(name="pOut", bufs=3) as pOut,
            tc.tile_pool(name="dram", bufs=1, space="DRAM") as dram,
        ):
            # ---- persistent SBUF ----
            xt_sb = const.tile([128, NKT * SQ], f16)
            for kt in range(NKT):
                nc.sync.dma_start(
                    xt_sb[:, kt * SQ:(kt + 1) * SQ],
                    xT[kt * 128:(kt + 1) * 128, :],
                )
            wq_sb = const.tile([128, D], f16)
            nc.sync.dma_start(wq_sb[:], wq[:, :])
            wk_sb = const.tile([128, D], f16)
            nc.sync.dma_start(wk_sb[:], wk[:, :])
            wv_sb = const.tile([128, D], f16)
            nc.sync.dma_start(wv_sb[:], wv[:, :])
            wo_sb = const.tile([128, D], f16)
            nc.sync.dma_start(wo_sb[:], wo[:, :])
            bq_sb = const.tile([128, 1], f32)
            nc.sync.dma_start(bq_sb[:], bq[:, :])
            bk_sb = const.tile([128, 1], f32)
            nc.sync.dma_start(bk_sb[:], bk[:, :])
            bvb_sb = const.tile([128, 128], f32)
            nc.sync.dma_start(bvb_sb[:], bvb[:, :])
            bo_sb = const.tile([128, 1], f32)
            nc.sync.dma_start(bo_sb[:], bo[:, :])

            ident = const.tile([128, 128], f16)
            make_identity(nc, ident)

            qT_sb = const.tile([128, SQ], f16)
            kT_sb = const.tile([128, SQ], f16)
            # V with ones column: per (b, head, key-tile) a [128, 65] region
            NREG = B * HPC * KT_S  # 64 regions
            vaug_sb = const.tile([128, NREG * 65], f16)
            # ones column for the softmax denominator
            ones_cols = vaug_sb.rearrange("p (r c) -> p r c", c=65)[:, :, 64:65]
            nc.vector.memset(ones_cols, 1.0)

            ctxT_sb = const.tile([128, SQ], f16)

            # ---- QKV projections ----
            for b in range(B):
                for (w_sb, b_sb, dst) in ((wq_sb, bq_sb, qT_sb), (wk_sb, bk_sb, kT_sb)):
                    for ncx in range(QC_S):
                        tok0 = b * S + ncx * 512
                        ps = psA.tile([128, 512], f32, tag="a")
                        for kt in range(NKT):
                            nc.tensor.matmul(
                                ps[:],
                                lhsT=w_sb[:, kt * 128:(kt + 1) * 128],
                                rhs=xt_sb[:, kt * SQ + tok0: kt * SQ + tok0 + 512],
                                start=(kt == 0),
                                stop=(kt == NKT - 1),
                            )
                        nc.scalar.activation(
                            dst[:, tok0:tok0 + 512], ps[:], AF.Identity,
                            bias=b_sb[:, 0:1],
                        )
                # V directly in [tokens, dims] layout
                for tt in range(KT_S):
                    tok0 = b * S + tt * 128
                    psv = psA.tile([128, 128], f32, tag="a")
                    for kt in range(NKT):
                        nc.tensor.matmul(
                            psv[:],
                            lhsT=xt_sb[:, kt * SQ + tok0: kt * SQ + tok0 + 128],
                            rhs=wv_sb[:, kt * 128:(kt + 1) * 128],
                            start=(kt == 0),
                            stop=(kt == NKT - 1),
                        )
                    for h in range(HPC):
                        r = (b * HPC + h) * KT_S + tt
                        nc.vector.tensor_add(
                            vaug_sb[:, r * 65: r * 65 + 64],
                            psv[:, h * 64:(h + 1) * 64],
                            bvb_sb[:, h * 64:(h + 1) * 64],
                        )

            # ---- attention ----
            for b in range(B):
                for qc in range(QC_S):
                    q0 = b * S + qc * 512
                    ctx_ps = [
                        psCtx.tile([128, 4 * 65], f32, tag="ctx", name=f"ctx_{b}_{qc}_{h}")
                        for h in range(HPC)
                    ]
                    for kt in range(KT_S):
                        k0 = b * S + kt * 128
                        sc = psSc.tile([128, 1024], f32, tag="sc", name=f"sc_{b}_{qc}_{kt}")
                        for h in range(HPC):
                            nc.tensor.matmul(
                                sc[:, h * 512:(h + 1) * 512],
                                lhsT=kT_sb[h * 64:(h + 1) * 64, k0:k0 + 128],
                                rhs=qT_sb[h * 64:(h + 1) * 64, q0:q0 + 512],
                                start=True,
                                stop=True,
                                tile_position=(h * 64, 0),
                            )
                        p_sb = pP.tile([128, 1024], f16, tag="p", name=f"p_{b}_{qc}_{kt}")
                        nc.scalar.activation(p_sb[:], sc[:], AF.Exp, scale=0.125)
                        for h in range(HPC):
                            r = (b * HPC + h) * KT_S + kt
                            for qs in range(4):
                                nc.tensor.matmul(
                                    ctx_ps[h][:, qs * 65:(qs + 1) * 65],
                                    lhsT=p_sb[:, h * 512 + qs * 128: h * 512 + (qs + 1) * 128],
                                    rhs=vaug_sb[:, r * 65:(r + 1) * 65],
                                    start=(kt == 0 and qs == 0),
                                    stop=(kt == KT_S - 1 and qs == 3),
                                )
                    # normalize + transpose into ctxT
                    for h in range(HPC):
                        for qs in range(4):
                            tok0 = q0 + qs * 128
                            rec = pSmall.tile([128, 1], f32, tag="rec", name=f"rec_{b}_{qc}_{h}_{qs}")
                            nc.vector.reciprocal(rec[:], ctx_ps[h][:, qs * 65 + 64: qs * 65 + 65])
                            ctxn = pCtxn.tile([128, 64], f16, tag="ctxn", name=f"ctxn_{b}_{qc}_{h}_{qs}")
                            nc.vector.tensor_scalar_mul(
                                ctxn[:], ctx_ps[h][:, qs * 65: qs * 65 + 64], rec[:],
                            )
                            trp = psA.tile([64, 128], f16, tag="a", name=f"trp_{b}_{qc}_{h}_{qs}")
                            nc.tensor.transpose(trp[:], ctxn[:], ident[:])
                            nc.vector.tensor_copy(
                                ctxT_sb[h * 64:(h + 1) * 64, tok0:tok0 + 128], trp[:],
                            )

            # ---- AllGather ctx^T across cores ----
            ctx_loc = dram.tile([128, SQ], f16)
            nc.sync.dma_start(ctx_loc[:], ctxT_sb[:])
            ctx_gath = dram.tile([NCORES * 128, SQ], f16, addr_space="Shared")
            nc.gpsimd.collective_compute(
                "AllGather",
                mybir.AluOpType.bypass,
                replica_groups=[list(range(NCORES))],
                ins=[ctx_loc.opt()],
                outs=[ctx_gath.opt()],
            )

            # ---- output projection (this core's 128 output columns) ----
            for ncx in range(SQ // 512):
                tok0 = ncx * 512
                ps = psA.tile([128, 512], f32, tag="a", name=f"op_{ncx}")
                for kt in range(NKT):
                    cg = pOut.tile([128, 512], f16, tag="cg", name=f"cg_{ncx}_{kt}")
                    nc.sync.dma_start(
                        cg[:], ctx_gath[kt * 128:(kt + 1) * 128, tok0:tok0 + 512],
                    )
                    nc.tensor.matmul(
                        ps[:],
                        lhsT=wo_sb[:, kt * 128:(kt + 1) * 128],
                        rhs=cg[:],
                        start=(kt == 0),
                        stop=(kt == NKT - 1),
                    )
                out_sb = pOut.tile([128, 512], f32, tag="os", name=f"os_{ncx}")
                nc.vector.tensor_scalar_add(out_sb[:], ps[:], bo_sb[:, 0:1])
                nc.sync.dma_start(yT[:, tok0:tok0 + 512], out_sb[:])

    nc.finalize()
    return nc


def kernel(x, Wq, Wk, Wv, bq, bk, bv, Wo, bo):
    from concourse.bass_utils import run_bass_kernel_spmd

    if "nc" not in _CACHE:
        _CACHE["nc"] = _build_kernel()
    nc = _CACHE["nc"]

    # host-side prep
    xTh = np.ascontiguousarray(
        x.astype(np.float32).transpose(2, 0, 1).reshape(D, SQ)
    ).astype(np.float16)

    def pack_w(Wslice):
        # [D, 128] -> [128, D] kt-major: out[p, kt*128+m] = Wslice[kt*128+p, m]
        return np.ascontiguousarray(
            Wslice.reshape(NKT, 128, 128).transpose(1, 0, 2).reshape(128, D)
        ).astype(np.float16)

    in_maps = []
    for c in range(NCORES):
        hA, hB = HPC * c, HPC * c + 1
        wq_c = pack_w(np.concatenate([Wq[hA], Wq[hB]], axis=1))
        wk_c = pack_w(np.concatenate([Wk[hA], Wk[hB]], axis=1))
        wv_c = pack_w(np.concatenate([Wv[hA], Wv[hB]], axis=1))
        wo_c = pack_w(Wo[:, 128 * c:128 * (c + 1)])
        bq_c = np.concatenate([bq[hA], bq[hB]]).reshape(128, 1).astype(np.float32)
        bk_c = np.concatenate([bk[hA], bk[hB]]).reshape(128, 1).astype(np.float32)
        bv_c = np.tile(
            np.concatenate([bv[hA], bv[hB]]).reshape(1, 128), (128, 1)
        ).astype(np.float32)
        bo_c = bo[128 * c:128 * (c + 1)].reshape(128, 1).astype(np.float32)
        in_maps.append({
            "xT": xTh, "wq": wq_c, "wk": wk_c, "wv": wv_c, "wo": wo_c,
            "bq": bq_c, "bk": bk_c, "bvb": bv_c, "bo": bo_c,
        })

    res = run_bass_kernel_spmd(nc, in_maps, core_ids=list(range(NCORES)))
    _CACHE["last_result"] = res
    # assemble: core c's yT [128, SQ] are output columns 128c..128c+127 (transposed)
    out = np.empty((B, S, D), dtype=np.float32)
    for c in range(NCORES):
        yt = res.results[c]["yT"]  # [128, SQ]
        out[:, :, 128 * c:128 * (c + 1)] = (
            yt.reshape(128, B, S).transpose(1, 2, 0)
        )
    return out
